# revision 37
# baseline (speedup 1.0000x reference)
"""Trainium2 Bass kernel for single-token-decode MHA with KV cache.

Problem: N=16, H=16, T0=4096, DQK=DV=128, DIM_IN=2048, fp32 inputs.
Sharding: head (tensor) parallelism across 8 cores — 2 heads per core, all
batches. Each core computes its 2 heads' attention plus the partial w_o
projection; the host sums the 8 partials (the "all-reduce after w_o").

The kernel is HBM-bound; optimization = fewer HBM bytes + tight streaming:
  - Weights/input fp16; K cache split by sequence chunk: first MK=12 of 32
    chunks fp16, the rest fp8 e3m4 (Trainium FP8_EXP3, 4 mantissa bits);
    V cache entirely e3m4. Host-side numpy sim of the exact pipeline puts
    rel err at 1.59e-2 (gate 2e-2; HW tracks the sim to ~1e-6).
  - Per-core HBM: 22.5 MB K + 16 MB V + 4.3 MB weights+input = 42.8 MB
    -> ~120 us floor at the ~358 GB/s per-NC HBM limit.
  - Transfer sizes kept at the DMA knee: K hi+lo bytes for a 2-pair couple
    ride ONE 1.41 MB byte-blob DMA (fp16 chunk views via AP.bitcast), V
    for the couple is one 1 MB DMA.
  - Queue plan: sync (HWDGE) + gpsimd (SWDGE) stream KV, greedily
    byte-balanced; the scalar engine's queue carries only the 4.3 MB of
    weights at the head, so its exp-activation semaphore waits can never
    starve a KV ring.
  - The new-token (k_new/v_new) path is folded into the per-head epilogue
    (DVE product + ones-column PE sum -> exp), so the per-pair stream
    depends only on w_q/input and the K/V tiles.
  - w_o stage runs transposed: out^T[c,n] accumulates per head in PSUM via
    lhsT=w_o chunks, rhs=y*(1/den); one [128, 256] copy + single output
    DMA at the end.
Compute per pair: 32 chunked QK^T matmuls (K chunk as lhsT, fp16 or e3m4
per chunk), exp+accum on ACT, 32 chained PV matmuls (V e3m4 lhsT x fp16
attn rhs), all accumulation in fp32 PSUM.
"""

import math

import numpy as np
import ml_dtypes

import concourse.bacc as bacc
import concourse.mybir as mybir
import concourse.tile as tile
from concourse.bass_utils import run_bass_kernel_spmd

N, H, T0, D, C = 16, 16, 4096, 128, 2048
NCORES = 8
HPC = H // NCORES          # heads per core = 2
TC = T0 // 128             # 32 sequence chunks of 128
MK = 6                     # K chunks kept in fp16 (rest e3m4)
ML = TC - MK
CCH = C // 128             # 16 contraction chunks of 128
P = HPC * N                # 32 (head, batch) pairs per core
G = P // 2                 # 16 pair-couples (K/V packed 2 pairs per DMA)
KPB = MK * 128 * 2 + ML * 128  # K bytes per pair per partition (5632)
SCALE = 1.0 / math.sqrt(D)

F32 = mybir.dt.float32
F16 = mybir.dt.float16
F8E3 = mybir.dt.float8e3

_CACHE: dict = {}


class _QueueBalancer:
    """Greedy compile-time byte balancing between the two KV DMA queues."""

    def __init__(self, nc):
        self.nc = nc
        self.bytes = {"sync": 0, "gpsimd": 0}

    def dma(self, out, in_, nbytes, queue=None):
        if queue is None:
            queue = min(self.bytes, key=self.bytes.get)
        if queue in self.bytes:
            self.bytes[queue] += nbytes
        eng = getattr(self.nc, queue)
        eng.dma_start(out=out, in_=in_)


def _build():
    if "nc" in _CACHE:
        return _CACHE["nc"]
    nc = bacc.Bacc(
        "TRN2",
        target_bir_lowering=False,
        debug=False,
        enable_asserts=False,
        num_devices=NCORES,
    )
    # K byte-blob per couple: per partition [pair0 hi f16 | pair0 lo e3m4 |
    # pair1 hi | pair1 lo]; hi = MK chunks fp16, lo = ML chunks e3m4.
    kb_d = nc.dram_tensor("kb", [G, 128, 2 * KPB], F8E3, kind="ExternalInput").ap()
    v_d = nc.dram_tensor("vx", [G, 128, 2 * TC * D], F8E3, kind="ExternalInput").ap()
    w_d = nc.dram_tensor("wqkv", [3, HPC, 128, CCH, D], F16, kind="ExternalInput").ap()
    wo_d = nc.dram_tensor("wo", [HPC, D, C], F16, kind="ExternalInput").ap()
    it_d = nc.dram_tensor("inpt", [128, CCH, N], F16, kind="ExternalInput").ap()
    out_d = nc.dram_tensor("out", [128, CCH, N], F32, kind="ExternalOutput").ap()

    with tile.TileContext(nc) as tc:
        with (
            tc.tile_pool(name="const", bufs=1) as const,
            tc.tile_pool(name="kv", bufs=4) as kvpool,
            tc.tile_pool(name="small", bufs=2) as small,
            tc.tile_pool(name="ypool", bufs=2) as ypool,
            tc.tile_pool(name="opool", bufs=1) as opool,
            tc.tile_pool(name="ps", bufs=2, space="PSUM") as ps,
            tc.tile_pool(name="wops", bufs=1, space="PSUM") as wops,
        ):
            ones_col = const.tile([128, 1], F32)
            nc.vector.memset(ones_col[:], 1.0)
            ones_row = const.tile([1, 128], F32)
            nc.vector.memset(ones_row[:], 1.0)

            qb = _QueueBalancer(nc)
            WB = 128 * CCH * D * 2  # one w chunk [128, CCH, D] fp16

            # w_q + input lead on the KV queues so q projections start
            # ASAP; the rest of the weights ride the scalar (ACT) HWDGE
            # ring — issued before any exp instruction exists, so they
            # never starve a KV ring.
            # K rides sync alone; V + weights ride gpsimd (byte-balanced
            # ~20.5 MB each) so the two issue streams' pool-waits are
            # decorrelated — one parked engine can't dry the other ring.
            w_sb = const.tile([128, HPC, 3, CCH, D], F16)
            inpt_sb = const.tile([128, CCH, N], F16)
            wo_sb = const.tile([128, HPC, C], F16)
            for h in range(HPC):
                qb.dma(w_sb[:, h, 0], w_d[0, h], WB, queue="gpsimd")
            qb.dma(inpt_sb[:], it_d, 128 * CCH * N * 2, queue="gpsimd")

            def _late_weights(g):
                if g == 1:
                    for h in range(HPC):
                        qb.dma(w_sb[:, h, 1], w_d[1, h], WB, queue="gpsimd")
                elif g == 2:
                    for h in range(HPC):
                        qb.dma(w_sb[:, h, 2], w_d[2, h], WB, queue="gpsimd")
                elif g == 3:
                    for h in range(HPC):
                        qb.dma(
                            wo_sb[:, h, :], wo_d[h], 128 * C * 2,
                            queue="gpsimd",
                        )

            # q projections only; k_new/v_new projections run in the head
            # epilogues so the per-pair stream never waits on w_k/w_v.
            def _proj(h, w, dt, tag):
                # allocated [128, 2N] so every "pm"-tag tile has one size
                # (bc_ps/sn_ps reuse the tag at [128, 2N])
                pp = ps.tile([128, 2 * N], F32, tag="pm", bufs=1, name="pp")
                for cc in range(CCH):
                    nc.tensor.matmul(
                        pp[:, :N],
                        lhsT=w_sb[:, h, w, cc, :],
                        rhs=inpt_sb[:, cc, :],
                        start=(cc == 0),
                        stop=(cc == CCH - 1),
                    )
                sb = small.tile([128, N], dt, tag=tag, name=tag)
                nc.vector.tensor_copy(out=sb[:], in_=pp[:, :N])
                return sb

            qTs = [_proj(h, 0, F16, f"q{h}") for h in range(HPC)]

            wo_acc2 = wops.tile([128, HPC, CCH, N], F32, tag="woa", name="woa")
            wo_acc = [wo_acc2[:, h] for h in range(HPC)]
            kv_tiles: dict = {}
            heads = [
                {
                    "qT": qTs[h],
                    "den": ps.tile(
                        [1, N], F32, tag=f"den{h}", bufs=1, name=f"den{h}"
                    ),
                    "y": ypool.tile([128, N], F32, tag=f"y{h}", name=f"y{h}"),
                }
                for h in range(HPC)
            ]

            def _phase_a(p):
                """DMA + QK + exp issue for pair p; returns state for B."""
                h, n = divmod(p, N)
                g, gi = p // 2, p % 2
                qT_sb = heads[h]["qT"]

                if g < G - 1:
                    if gi == 0:
                        kraw = kvpool.tile(
                            [128, 2 * KPB], F8E3, tag="kraw", bufs=4,
                            name="kraw",
                        )
                        qb.dma(kraw[:], kb_d[g], 128 * 2 * KPB, queue="sync")
                        v2 = kvpool.tile(
                            [128, 2, TC, D], F8E3, tag="v8", bufs=3, name="v8"
                        )
                        qb.dma(
                            v2[:], v_d[g], 128 * 2 * TC * D, queue="gpsimd"
                        )
                        _late_weights(g)
                        kv_tiles[g] = (kraw, v2)
                    kraw, v_sb = kv_tiles[g]
                    kbase, vsl = gi * KPB, gi
                else:
                    # last couple: per-pair tiles/DMAs so the final pair's
                    # compute overlaps the final pair's bytes
                    kraw = kvpool.tile(
                        [128, KPB], F8E3, tag="krawL", bufs=2, name="krawL"
                    )
                    qb.dma(
                        kraw[:], kb_d[g, :, gi * KPB : (gi + 1) * KPB],
                        128 * KPB, queue="sync",
                    )
                    v_sb = kvpool.tile(
                        [128, 1, TC, D], F8E3, tag="v8L", bufs=2, name="v8L"
                    )
                    qb.dma(
                        v_sb[:],
                        v_d[g, :, gi * TC * D : (gi + 1) * TC * D],
                        128 * TC * D, queue="gpsimd",
                    )
                    kbase, vsl = 0, 0

                sc = ps.tile([128, TC], F32, tag="sc")
                for c in range(TC):
                    if c < MK:
                        off = kbase + c * 256
                        lhs = kraw[:, off : off + 256].bitcast(F16)
                    else:
                        off = kbase + MK * 256 + (c - MK) * 128
                        lhs = kraw[:, off : off + 128]
                    nc.tensor.matmul(
                        sc[:, c : c + 1],
                        lhsT=lhs,
                        rhs=qT_sb[:, n : n + 1],
                        start=True,
                        stop=True,
                    )

                attn = small.tile([128, TC], F16, tag="attn")
                acc = small.tile([128, 1], F32, tag="acc")
                nc.scalar.activation(
                    out=attn[:],
                    in_=sc[:],
                    func=mybir.ActivationFunctionType.Exp,
                    scale=SCALE,
                    accum_out=acc[:],
                )
                return (h, n, attn, acc, v_sb, vsl)

            def _phase_b(state):
                """den + PV + y copy for a pair whose exp already issued."""
                h, n, attn, acc, v_sb, vsl = state
                nc.tensor.matmul(
                    heads[h]["den"][0:1, n : n + 1],
                    lhsT=ones_col[:],
                    rhs=acc[:],
                    start=True,
                    stop=True,
                )
                y_ps = ps.tile([128, 1], F32, tag="yps")
                for c in range(TC):
                    nc.tensor.matmul(
                        y_ps[:],
                        lhsT=v_sb[:, vsl, c, :],
                        rhs=attn[:, c : c + 1],
                        start=(c == 0),
                        stop=(c == TC - 1),
                    )
                nc.vector.tensor_copy(
                    out=heads[h]["y"][:, n : n + 1], in_=y_ps[:]
                )
                if n == N - 1:
                    _epilogue(h)

            projs_kv: dict = {}

            def _epilogue(h):
                qT_sb = heads[h]["qT"]
                den_ps = heads[h]["den"]
                y_sb = heads[h]["y"]
                knT_sb, vnT_sb = projs_kv[h]

                # s_new[n] = k_new[:,n] . q[:,n] via DVE product + ones sum
                prod = ypool.tile([128, N], F32, tag="prod")
                nc.vector.tensor_mul(out=prod[:], in0=knT_sb[:], in1=qT_sb[:])
                sn_ps = ps.tile([128, 2 * N], F32, tag="pm", bufs=1, name="snps")
                nc.tensor.matmul(
                    sn_ps[0:1, :N], lhsT=ones_col[:], rhs=prod[:],
                    start=True, stop=True,
                )
                en_sb = small.tile([1, N], F32, tag="en")
                nc.scalar.activation(
                    out=en_sb[:],
                    in_=sn_ps[0:1, :N],
                    func=mybir.ActivationFunctionType.Exp,
                    scale=SCALE,
                )
                # den_tot = den + exp_new; inv = 1/den_tot; eninv = en*inv
                den_sb = small.tile([1, N], F32, tag="densb")
                nc.vector.tensor_copy(out=den_sb[:], in_=den_ps[:])
                nc.vector.tensor_add(out=den_sb[:], in0=den_sb[:], in1=en_sb[:])
                inv_sb = small.tile([1, N], F32, tag="inv")
                nc.vector.reciprocal(inv_sb[:], den_sb[:])
                eninv = small.tile([1, N], F32, tag="eninv")
                nc.vector.tensor_mul(out=eninv[:], in0=en_sb[:], in1=inv_sb[:])
                # broadcast rows across partitions; y_fin = y*inv + vn*eninv
                bc_ps = ps.tile([128, 2 * N], F32, tag="pm", bufs=1, name="bcps")
                nc.tensor.matmul(
                    bc_ps[:, :N], lhsT=ones_row[:], rhs=inv_sb[:],
                    start=True, stop=True,
                )
                nc.tensor.matmul(
                    bc_ps[:, N:], lhsT=ones_row[:], rhs=eninv[:],
                    start=True, stop=True,
                )
                tmp_y = ypool.tile([128, N], F32, tag="tmpy")
                nc.vector.tensor_mul(out=tmp_y[:], in0=vnT_sb[:], in1=bc_ps[:, N:])
                nc.vector.tensor_mul(out=y_sb[:], in0=y_sb[:], in1=bc_ps[:, :N])
                nc.vector.tensor_add(out=y_sb[:], in0=y_sb[:], in1=tmp_y[:])
                y16 = ypool.tile([128, N], F16, tag="y16")
                nc.vector.tensor_copy(out=y16[:], in_=y_sb[:])

                # out^T partial: wo_acc[h][c, n] = sum_d w_o[h, d, c] y16[d, n]
                for cc in range(CCH):
                    nc.tensor.matmul(
                        wo_acc[h][:, cc, :],
                        lhsT=wo_sb[:, h, cc * 128 : (cc + 1) * 128],
                        rhs=y16[:],
                        start=True,
                        stop=True,
                    )

            # software-pipelined: QK/exp of pair p+1 is emitted before PV of
            # pair p, so the PE stream never stalls on pair p's exp latency
            prev = None
            for p in range(P):
                state = _phase_a(p)
                if prev is not None:
                    _phase_b(prev)
                prev = state
                # k_new/v_new projections mid-stream (weights have landed,
                # PE has slack) so the h1 epilogue isn't gated on them
                if p == 8:
                    for h in range(HPC):
                        projs_kv[h] = [_proj(h, 1, F16, f"kn{h}")]
                elif p == 10:
                    for h in range(HPC):
                        projs_kv[h].append(_proj(h, 2, F32, f"vn{h}"))
            _phase_b(prev)

            outT = opool.tile([128, CCH, N], F32)
            nc.vector.tensor_copy(out=outT[:], in_=wo_acc[0])
            nc.vector.tensor_add(out=outT[:], in0=outT[:], in1=wo_acc[1])
            qb.dma(out_d, outT[:], 128 * CCH * N * 4, queue="scalar")

    nc.compile()
    _CACHE["nc"] = nc
    return nc


def shard_inputs(input, k_cache, v_cache, w_q, w_k, w_v, w_o):
    """Host-side layout/dtype prep: per-core input dicts."""
    input = np.asarray(input, dtype=np.float32)
    k_cache = np.asarray(k_cache, dtype=np.float32)
    v_cache = np.asarray(v_cache, dtype=np.float32)
    w_q = np.asarray(w_q, dtype=np.float32)
    w_k = np.asarray(w_k, dtype=np.float32)
    w_v = np.asarray(w_v, dtype=np.float32)
    w_o = np.asarray(w_o, dtype=np.float32)

    inpT = input.reshape(N, C).T  # [C, N]
    it_np = np.ascontiguousarray(
        inpT.reshape(CCH, 128, N).transpose(1, 0, 2)
    ).astype(np.float16)
    wo4 = w_o.reshape(H, D, C)
    wqkv = np.stack([w_q, w_k, w_v])  # [3, H, D, C]

    in_maps = []
    for core in range(NCORES):
        h0 = core * HPC
        # K^T per pair p = h*N + n: [P, D, T0]; blob per couple/partition:
        # [pair0 hi f16 | pair0 lo e3m4 | pair1 hi | pair1 lo]
        kT = k_cache[:, h0 : h0 + HPC].transpose(1, 0, 3, 2).reshape(P, D, T0)
        kh = np.ascontiguousarray(kT[:, :, : MK * 128]).astype(np.float16)
        kl = np.ascontiguousarray(kT[:, :, MK * 128 :]).astype(
            ml_dtypes.float8_e3m4
        )
        kb = np.empty((P, D, KPB), dtype=np.uint8)
        kb[:, :, : MK * 256] = kh.view(np.uint8)
        kb[:, :, MK * 256 :] = kl.view(np.uint8)
        kb = (
            kb.reshape(G, 2, D, KPB)
            .transpose(0, 2, 1, 3)
            .reshape(G, 128, 2 * KPB)
        )
        kb = np.ascontiguousarray(kb).view(ml_dtypes.float8_e3m4)
        # V packed per couple: [G, 128, 2, TC, D] e3m4 where
        # [g, pp, i, c, j] = V_{p=2g+i}[c*128+pp, j]
        v_np = (
            v_cache[:, h0 : h0 + HPC]
            .transpose(1, 0, 2, 3)            # [HPC, N, T0, DV]
            .reshape(P, TC, 128, D)
            .transpose(0, 2, 1, 3)            # [P, 128, TC, D]
            .reshape(G, 2, 128, TC, D)
            .transpose(0, 2, 1, 3, 4)         # [G, 128, 2, TC, D]
            .reshape(G, 128, 2 * TC * D)
        )
        v_np = np.ascontiguousarray(v_np).astype(ml_dtypes.float8_e3m4)
        # wT chunks: [3, HPC, 128, CCH, D]; wT[h] = w[h].T of shape [C, D]
        w_np = np.ascontiguousarray(
            wqkv[:, h0 : h0 + HPC]
            .transpose(0, 1, 3, 2)  # [3, HPC, C, D]
            .reshape(3, HPC, CCH, 128, D)
            .transpose(0, 1, 3, 2, 4)
        ).astype(np.float16)  # [3, HPC, 128, CCH, D]
        wo_np = np.ascontiguousarray(wo4[h0 : h0 + HPC]).astype(np.float16)
        in_maps.append(
            {
                "kb": kb,
                "vx": v_np,
                "wqkv": w_np,
                "wo": wo_np,
                "inpt": it_np,
            }
        )
    return in_maps


def _run(inputs: dict, trace: bool = False):
    nc = _build()
    in_maps = shard_inputs(**inputs)
    res = run_bass_kernel_spmd(
        nc, in_maps, core_ids=list(range(NCORES)), trace=trace
    )
    # out DRAM is out^T chunks: [128, CCH, N] with c = cc*128 + p
    acc = np.zeros((N, C), dtype=np.float64)
    for r in res.results:
        o = r["out"].reshape(128, CCH, N)
        acc += o.transpose(2, 1, 0).reshape(N, C)
    out = acc.astype(np.float32).reshape(N, 1, C)
    return out, res


def kernel(**inputs) -> np.ndarray:
    out, _ = _run(inputs, trace=False)
    return out


# revision 39
# speedup vs baseline: 1.0022x; 1.0022x over previous
"""Trainium2 Bass kernel for single-token-decode MHA with KV cache.

Problem: N=16, H=16, T0=4096, DQK=DV=128, DIM_IN=2048, fp32 inputs.
Sharding: head (tensor) parallelism across 8 cores — 2 heads per core, all
batches. Each core computes its 2 heads' attention plus the partial w_o
projection; the host sums the 8 partials (the "all-reduce after w_o").

The kernel is HBM-bound; optimization = fewer HBM bytes + tight streaming:
  - Weights/input fp16; K cache split by sequence chunk: first MK=12 of 32
    chunks fp16, the rest fp8 e3m4 (Trainium FP8_EXP3, 4 mantissa bits);
    V cache entirely e3m4. Host-side numpy sim of the exact pipeline puts
    rel err at 1.59e-2 (gate 2e-2; HW tracks the sim to ~1e-6).
  - Per-core HBM: 22.5 MB K + 16 MB V + 4.3 MB weights+input = 42.8 MB
    -> ~120 us floor at the ~358 GB/s per-NC HBM limit.
  - Transfer sizes kept at the DMA knee: K hi+lo bytes for a 2-pair couple
    ride ONE 1.41 MB byte-blob DMA (fp16 chunk views via AP.bitcast), V
    for the couple is one 1 MB DMA.
  - Queue plan: sync (HWDGE) + gpsimd (SWDGE) stream KV, greedily
    byte-balanced; the scalar engine's queue carries only the 4.3 MB of
    weights at the head, so its exp-activation semaphore waits can never
    starve a KV ring.
  - The new-token (k_new/v_new) path is folded into the per-head epilogue
    (DVE product + ones-column PE sum -> exp), so the per-pair stream
    depends only on w_q/input and the K/V tiles.
  - w_o stage runs transposed: out^T[c,n] accumulates per head in PSUM via
    lhsT=w_o chunks, rhs=y*(1/den); one [128, 256] copy + single output
    DMA at the end.
Compute per pair: 32 chunked QK^T matmuls (K chunk as lhsT, fp16 or e3m4
per chunk), exp+accum on ACT, 32 chained PV matmuls (V e3m4 lhsT x fp16
attn rhs), all accumulation in fp32 PSUM.
"""

import math

import numpy as np
import ml_dtypes

import concourse.bacc as bacc
import concourse.mybir as mybir
import concourse.tile as tile
from concourse.bass_utils import run_bass_kernel_spmd

N, H, T0, D, C = 16, 16, 4096, 128, 2048
NCORES = 8
HPC = H // NCORES          # heads per core = 2
TC = T0 // 128             # 32 sequence chunks of 128
MK = 6                     # K chunks kept in fp16 (rest e3m4)
ML = TC - MK
CCH = C // 128             # 16 contraction chunks of 128
P = HPC * N                # 32 (head, batch) pairs per core
G = P // 2                 # 16 pair-couples (K/V packed 2 pairs per DMA)
KPB = MK * 128 * 2 + ML * 128  # K bytes per pair per partition (5632)
SCALE = 1.0 / math.sqrt(D)

F32 = mybir.dt.float32
F16 = mybir.dt.float16
F8E3 = mybir.dt.float8e3

_CACHE: dict = {}


class _QueueBalancer:
    """Greedy compile-time byte balancing between the two KV DMA queues."""

    def __init__(self, nc):
        self.nc = nc
        self.bytes = {"sync": 0, "gpsimd": 0}

    def dma(self, out, in_, nbytes, queue=None):
        if queue is None:
            queue = min(self.bytes, key=self.bytes.get)
        if queue in self.bytes:
            self.bytes[queue] += nbytes
        eng = getattr(self.nc, queue)
        eng.dma_start(out=out, in_=in_)


def _build():
    if "nc" in _CACHE:
        return _CACHE["nc"]
    nc = bacc.Bacc(
        "TRN2",
        target_bir_lowering=False,
        debug=False,
        enable_asserts=False,
        num_devices=NCORES,
    )
    # K byte-blob per couple: per partition [pair0 hi f16 | pair0 lo e3m4 |
    # pair1 hi | pair1 lo]; hi = MK chunks fp16, lo = ML chunks e3m4.
    kb_d = nc.dram_tensor("kb", [G, 128, 2 * KPB], F8E3, kind="ExternalInput").ap()
    v_d = nc.dram_tensor("vx", [G, 128, 2 * TC * D], F8E3, kind="ExternalInput").ap()
    w_d = nc.dram_tensor("wqkv", [3, HPC, 128, CCH, D], F16, kind="ExternalInput").ap()
    wo_d = nc.dram_tensor("wo", [HPC, D, C], F16, kind="ExternalInput").ap()
    it_d = nc.dram_tensor("inpt", [128, CCH, N], F16, kind="ExternalInput").ap()
    out_d = nc.dram_tensor("out", [128, CCH, N], F32, kind="ExternalOutput").ap()

    with tile.TileContext(nc) as tc:
        with (
            tc.tile_pool(name="const", bufs=1) as const,
            tc.tile_pool(name="kv", bufs=4) as kvpool,
            tc.tile_pool(name="small", bufs=2) as small,
            tc.tile_pool(name="ypool", bufs=2) as ypool,
            tc.tile_pool(name="opool", bufs=1) as opool,
            tc.tile_pool(name="ps", bufs=2, space="PSUM") as ps,
            tc.tile_pool(name="wops", bufs=1, space="PSUM") as wops,
        ):
            ones_col = const.tile([128, 1], F32)
            nc.vector.memset(ones_col[:], 1.0)
            ones_row = const.tile([1, 128], F32)
            nc.vector.memset(ones_row[:], 1.0)

            qb = _QueueBalancer(nc)
            WB = 128 * CCH * D * 2  # one w chunk [128, CCH, D] fp16

            # w_q + input lead on the KV queues so q projections start
            # ASAP; the rest of the weights ride the scalar (ACT) HWDGE
            # ring — issued before any exp instruction exists, so they
            # never starve a KV ring.
            # K rides sync alone; V + weights ride gpsimd (byte-balanced
            # ~20.5 MB each) so the two issue streams' pool-waits are
            # decorrelated — one parked engine can't dry the other ring.
            w_sb = const.tile([128, HPC, 3, CCH, D], F16)
            inpt_sb = const.tile([128, CCH, N], F16)
            wo_sb = const.tile([128, HPC, C], F16)
            for h in range(HPC):
                qb.dma(w_sb[:, h, 0], w_d[0, h], WB, queue="gpsimd")
            qb.dma(inpt_sb[:], it_d, 128 * CCH * N * 2, queue="gpsimd")

            def _late_weights(g):
                if g == 1:
                    for h in range(HPC):
                        qb.dma(w_sb[:, h, 1], w_d[1, h], WB, queue="gpsimd")
                elif g == 2:
                    for h in range(HPC):
                        qb.dma(w_sb[:, h, 2], w_d[2, h], WB, queue="gpsimd")
                elif g == 3:
                    for h in range(HPC):
                        qb.dma(
                            wo_sb[:, h, :], wo_d[h], 128 * C * 2,
                            queue="gpsimd",
                        )

            # q projections only; k_new/v_new projections run in the head
            # epilogues so the per-pair stream never waits on w_k/w_v.
            def _proj(h, w, dt, tag):
                # allocated [128, 2N] so every "pm"-tag tile has one size
                # (bc_ps/sn_ps reuse the tag at [128, 2N])
                pp = ps.tile([128, 2 * N], F32, tag="pm", bufs=1, name="pp")
                for cc in range(CCH):
                    nc.tensor.matmul(
                        pp[:, :N],
                        lhsT=w_sb[:, h, w, cc, :],
                        rhs=inpt_sb[:, cc, :],
                        start=(cc == 0),
                        stop=(cc == CCH - 1),
                    )
                sb = small.tile([128, N], dt, tag=tag, name=tag)
                nc.vector.tensor_copy(out=sb[:], in_=pp[:, :N])
                return sb

            qTs = [_proj(h, 0, F16, f"q{h}") for h in range(HPC)]

            wo_acc2 = wops.tile([128, HPC, CCH, N], F32, tag="woa", name="woa")
            wo_acc = [wo_acc2[:, h] for h in range(HPC)]
            kv_tiles: dict = {}
            heads = [
                {
                    "qT": qTs[h],
                    "den": ps.tile(
                        [1, N], F32, tag=f"den{h}", bufs=1, name=f"den{h}"
                    ),
                    "y": ypool.tile([128, N], F32, tag=f"y{h}", name=f"y{h}"),
                }
                for h in range(HPC)
            ]

            def _phase_a(p):
                """DMA + QK + exp issue for pair p; returns state for B."""
                h, n = divmod(p, N)
                g, gi = p // 2, p % 2
                qT_sb = heads[h]["qT"]

                if g < G - 1:
                    if gi == 0:
                        kraw = kvpool.tile(
                            [128, 2 * KPB], F8E3, tag="kraw", bufs=4,
                            name="kraw",
                        )
                        qb.dma(kraw[:], kb_d[g], 128 * 2 * KPB, queue="sync")
                        v2 = kvpool.tile(
                            [128, 2, TC, D], F8E3, tag="v8", bufs=3, name="v8"
                        )
                        qb.dma(
                            v2[:], v_d[g], 128 * 2 * TC * D, queue="gpsimd"
                        )
                        _late_weights(g)
                        kv_tiles[g] = (kraw, v2)
                    kraw, v_sb = kv_tiles[g]
                    kbase, vsl = gi * KPB, gi
                else:
                    # last couple: per-pair tiles with HALF-tile DMAs so
                    # the final pairs' compute overlaps the final bytes
                    kraw = kvpool.tile(
                        [128, KPB], F8E3, tag="krawL", bufs=2, name="krawL"
                    )
                    hk = KPB // 2
                    for i in range(2):
                        qb.dma(
                            kraw[:, i * hk : (i + 1) * hk],
                            kb_d[g, :, gi * KPB + i * hk : gi * KPB + (i + 1) * hk],
                            128 * hk, queue="sync",
                        )
                    v_sb = kvpool.tile(
                        [128, 1, TC, D], F8E3, tag="v8L", bufs=2, name="v8L"
                    )
                    hv = TC * D // 2
                    for i in range(2):
                        qb.dma(
                            v_sb[:, 0, i * (TC // 2) : (i + 1) * (TC // 2), :],
                            v_d[g, :, gi * TC * D + i * hv : gi * TC * D + (i + 1) * hv],
                            128 * hv, queue="gpsimd",
                        )
                    kbase, vsl = 0, 0

                sc = ps.tile([128, TC], F32, tag="sc")
                for c in range(TC):
                    if c < MK:
                        off = kbase + c * 256
                        lhs = kraw[:, off : off + 256].bitcast(F16)
                    else:
                        off = kbase + MK * 256 + (c - MK) * 128
                        lhs = kraw[:, off : off + 128]
                    nc.tensor.matmul(
                        sc[:, c : c + 1],
                        lhsT=lhs,
                        rhs=qT_sb[:, n : n + 1],
                        start=True,
                        stop=True,
                    )

                attn = small.tile([128, TC], F16, tag="attn")
                acc = small.tile([128, 1], F32, tag="acc")
                nc.scalar.activation(
                    out=attn[:],
                    in_=sc[:],
                    func=mybir.ActivationFunctionType.Exp,
                    scale=SCALE,
                    accum_out=acc[:],
                )
                return (h, n, attn, acc, v_sb, vsl)

            def _phase_b(state):
                """den + PV + y copy for a pair whose exp already issued."""
                h, n, attn, acc, v_sb, vsl = state
                nc.tensor.matmul(
                    heads[h]["den"][0:1, n : n + 1],
                    lhsT=ones_col[:],
                    rhs=acc[:],
                    start=True,
                    stop=True,
                )
                y_ps = ps.tile([128, 1], F32, tag="yps")
                for c in range(TC):
                    nc.tensor.matmul(
                        y_ps[:],
                        lhsT=v_sb[:, vsl, c, :],
                        rhs=attn[:, c : c + 1],
                        start=(c == 0),
                        stop=(c == TC - 1),
                    )
                nc.vector.tensor_copy(
                    out=heads[h]["y"][:, n : n + 1], in_=y_ps[:]
                )
                if n == N - 1:
                    _epilogue(h)

            projs_kv: dict = {}

            def _epilogue(h):
                qT_sb = heads[h]["qT"]
                den_ps = heads[h]["den"]
                y_sb = heads[h]["y"]
                knT_sb, vnT_sb = projs_kv[h]

                # s_new[n] = k_new[:,n] . q[:,n] via DVE product + ones sum
                prod = ypool.tile([128, N], F32, tag="prod")
                nc.vector.tensor_mul(out=prod[:], in0=knT_sb[:], in1=qT_sb[:])
                sn_ps = ps.tile([128, 2 * N], F32, tag="pm", bufs=1, name="snps")
                nc.tensor.matmul(
                    sn_ps[0:1, :N], lhsT=ones_col[:], rhs=prod[:],
                    start=True, stop=True,
                )
                en_sb = small.tile([1, N], F32, tag="en")
                nc.scalar.activation(
                    out=en_sb[:],
                    in_=sn_ps[0:1, :N],
                    func=mybir.ActivationFunctionType.Exp,
                    scale=SCALE,
                )
                # den_tot = den + exp_new; inv = 1/den_tot; eninv = en*inv
                den_sb = small.tile([1, N], F32, tag="densb")
                nc.vector.tensor_copy(out=den_sb[:], in_=den_ps[:])
                nc.vector.tensor_add(out=den_sb[:], in0=den_sb[:], in1=en_sb[:])
                inv_sb = small.tile([1, N], F32, tag="inv")
                nc.vector.reciprocal(inv_sb[:], den_sb[:])
                eninv = small.tile([1, N], F32, tag="eninv")
                nc.vector.tensor_mul(out=eninv[:], in0=en_sb[:], in1=inv_sb[:])
                # broadcast rows across partitions; y_fin = y*inv + vn*eninv
                bc_ps = ps.tile([128, 2 * N], F32, tag="pm", bufs=1, name="bcps")
                nc.tensor.matmul(
                    bc_ps[:, :N], lhsT=ones_row[:], rhs=inv_sb[:],
                    start=True, stop=True,
                )
                nc.tensor.matmul(
                    bc_ps[:, N:], lhsT=ones_row[:], rhs=eninv[:],
                    start=True, stop=True,
                )
                tmp_y = ypool.tile([128, N], F32, tag="tmpy")
                nc.vector.tensor_mul(out=tmp_y[:], in0=vnT_sb[:], in1=bc_ps[:, N:])
                nc.vector.tensor_mul(out=y_sb[:], in0=y_sb[:], in1=bc_ps[:, :N])
                nc.vector.tensor_add(out=y_sb[:], in0=y_sb[:], in1=tmp_y[:])
                y16 = ypool.tile([128, N], F16, tag="y16")
                nc.vector.tensor_copy(out=y16[:], in_=y_sb[:])

                # out^T partial: wo_acc[h][c, n] = sum_d w_o[h, d, c] y16[d, n]
                for cc in range(CCH):
                    nc.tensor.matmul(
                        wo_acc[h][:, cc, :],
                        lhsT=wo_sb[:, h, cc * 128 : (cc + 1) * 128],
                        rhs=y16[:],
                        start=True,
                        stop=True,
                    )

            for p in range(P):
                _phase_b(_phase_a(p))
                # k_new/v_new projections mid-stream (weights have landed,
                # PE has slack) so the h1 epilogue isn't gated on them
                if p == 8:
                    for h in range(HPC):
                        projs_kv[h] = [_proj(h, 1, F16, f"kn{h}")]
                elif p == 10:
                    for h in range(HPC):
                        projs_kv[h].append(_proj(h, 2, F32, f"vn{h}"))

            outT = opool.tile([128, CCH, N], F32)
            nc.vector.tensor_copy(out=outT[:], in_=wo_acc[0])
            nc.vector.tensor_add(out=outT[:], in0=outT[:], in1=wo_acc[1])
            qb.dma(out_d, outT[:], 128 * CCH * N * 4, queue="scalar")

    nc.compile()
    _CACHE["nc"] = nc
    return nc


def shard_inputs(input, k_cache, v_cache, w_q, w_k, w_v, w_o):
    """Host-side layout/dtype prep: per-core input dicts."""
    input = np.asarray(input, dtype=np.float32)
    k_cache = np.asarray(k_cache, dtype=np.float32)
    v_cache = np.asarray(v_cache, dtype=np.float32)
    w_q = np.asarray(w_q, dtype=np.float32)
    w_k = np.asarray(w_k, dtype=np.float32)
    w_v = np.asarray(w_v, dtype=np.float32)
    w_o = np.asarray(w_o, dtype=np.float32)

    inpT = input.reshape(N, C).T  # [C, N]
    it_np = np.ascontiguousarray(
        inpT.reshape(CCH, 128, N).transpose(1, 0, 2)
    ).astype(np.float16)
    wo4 = w_o.reshape(H, D, C)
    wqkv = np.stack([w_q, w_k, w_v])  # [3, H, D, C]

    in_maps = []
    for core in range(NCORES):
        h0 = core * HPC
        # K^T per pair p = h*N + n: [P, D, T0]; blob per couple/partition:
        # [pair0 hi f16 | pair0 lo e3m4 | pair1 hi | pair1 lo]
        kT = k_cache[:, h0 : h0 + HPC].transpose(1, 0, 3, 2).reshape(P, D, T0)
        kh = np.ascontiguousarray(kT[:, :, : MK * 128]).astype(np.float16)
        kl = np.ascontiguousarray(kT[:, :, MK * 128 :]).astype(
            ml_dtypes.float8_e3m4
        )
        kb = np.empty((P, D, KPB), dtype=np.uint8)
        kb[:, :, : MK * 256] = kh.view(np.uint8)
        kb[:, :, MK * 256 :] = kl.view(np.uint8)
        kb = (
            kb.reshape(G, 2, D, KPB)
            .transpose(0, 2, 1, 3)
            .reshape(G, 128, 2 * KPB)
        )
        kb = np.ascontiguousarray(kb).view(ml_dtypes.float8_e3m4)
        # V packed per couple: [G, 128, 2, TC, D] e3m4 where
        # [g, pp, i, c, j] = V_{p=2g+i}[c*128+pp, j]
        v_np = (
            v_cache[:, h0 : h0 + HPC]
            .transpose(1, 0, 2, 3)            # [HPC, N, T0, DV]
            .reshape(P, TC, 128, D)
            .transpose(0, 2, 1, 3)            # [P, 128, TC, D]
            .reshape(G, 2, 128, TC, D)
            .transpose(0, 2, 1, 3, 4)         # [G, 128, 2, TC, D]
            .reshape(G, 128, 2 * TC * D)
        )
        v_np = np.ascontiguousarray(v_np).astype(ml_dtypes.float8_e3m4)
        # wT chunks: [3, HPC, 128, CCH, D]; wT[h] = w[h].T of shape [C, D]
        w_np = np.ascontiguousarray(
            wqkv[:, h0 : h0 + HPC]
            .transpose(0, 1, 3, 2)  # [3, HPC, C, D]
            .reshape(3, HPC, CCH, 128, D)
            .transpose(0, 1, 3, 2, 4)
        ).astype(np.float16)  # [3, HPC, 128, CCH, D]
        wo_np = np.ascontiguousarray(wo4[h0 : h0 + HPC]).astype(np.float16)
        in_maps.append(
            {
                "kb": kb,
                "vx": v_np,
                "wqkv": w_np,
                "wo": wo_np,
                "inpt": it_np,
            }
        )
    return in_maps


def _run(inputs: dict, trace: bool = False):
    nc = _build()
    in_maps = shard_inputs(**inputs)
    res = run_bass_kernel_spmd(
        nc, in_maps, core_ids=list(range(NCORES)), trace=trace
    )
    # out DRAM is out^T chunks: [128, CCH, N] with c = cc*128 + p
    acc = np.zeros((N, C), dtype=np.float64)
    for r in res.results:
        o = r["out"].reshape(128, CCH, N)
        acc += o.transpose(2, 1, 0).reshape(N, C)
    out = acc.astype(np.float32).reshape(N, 1, C)
    return out, res


def kernel(**inputs) -> np.ndarray:
    out, _ = _run(inputs, trace=False)
    return out


# revision 40
# speedup vs baseline: 1.0061x; 1.0039x over previous
"""Trainium2 Bass kernel for single-token-decode MHA with KV cache.

Problem: N=16, H=16, T0=4096, DQK=DV=128, DIM_IN=2048, fp32 inputs.
Sharding: head (tensor) parallelism across 8 cores — 2 heads per core, all
batches. Each core computes its 2 heads' attention plus the partial w_o
projection; the host sums the 8 partials (the "all-reduce after w_o").

The kernel is HBM-bound; optimization = fewer HBM bytes + tight streaming:
  - Weights/input fp16; K cache split by sequence chunk: first MK=12 of 32
    chunks fp16, the rest fp8 e3m4 (Trainium FP8_EXP3, 4 mantissa bits);
    V cache entirely e3m4. Host-side numpy sim of the exact pipeline puts
    rel err at 1.59e-2 (gate 2e-2; HW tracks the sim to ~1e-6).
  - Per-core HBM: 22.5 MB K + 16 MB V + 4.3 MB weights+input = 42.8 MB
    -> ~120 us floor at the ~358 GB/s per-NC HBM limit.
  - Transfer sizes kept at the DMA knee: K hi+lo bytes for a 2-pair couple
    ride ONE 1.41 MB byte-blob DMA (fp16 chunk views via AP.bitcast), V
    for the couple is one 1 MB DMA.
  - Queue plan: sync (HWDGE) + gpsimd (SWDGE) stream KV, greedily
    byte-balanced; the scalar engine's queue carries only the 4.3 MB of
    weights at the head, so its exp-activation semaphore waits can never
    starve a KV ring.
  - The new-token (k_new/v_new) path is folded into the per-head epilogue
    (DVE product + ones-column PE sum -> exp), so the per-pair stream
    depends only on w_q/input and the K/V tiles.
  - w_o stage runs transposed: out^T[c,n] accumulates per head in PSUM via
    lhsT=w_o chunks, rhs=y*(1/den); one [128, 256] copy + single output
    DMA at the end.
Compute per pair: 32 chunked QK^T matmuls (K chunk as lhsT, fp16 or e3m4
per chunk), exp+accum on ACT, 32 chained PV matmuls (V e3m4 lhsT x fp16
attn rhs), all accumulation in fp32 PSUM.
"""

import math

import numpy as np
import ml_dtypes

import concourse.bacc as bacc
import concourse.mybir as mybir
import concourse.tile as tile
from concourse.bass_utils import run_bass_kernel_spmd

N, H, T0, D, C = 16, 16, 4096, 128, 2048
NCORES = 8
HPC = H // NCORES          # heads per core = 2
TC = T0 // 128             # 32 sequence chunks of 128
MK = 6                     # K chunks kept in fp16 (rest e3m4)
ML = TC - MK
CCH = C // 128             # 16 contraction chunks of 128
P = HPC * N                # 32 (head, batch) pairs per core
G = P // 2                 # 16 pair-couples (K/V packed 2 pairs per DMA)
KPB = MK * 128 * 2 + ML * 128  # K bytes per pair per partition (5632)
SCALE = 1.0 / math.sqrt(D)

F32 = mybir.dt.float32
F16 = mybir.dt.float16
F8E3 = mybir.dt.float8e3

_CACHE: dict = {}


class _QueueBalancer:
    """Greedy compile-time byte balancing between the two KV DMA queues."""

    def __init__(self, nc):
        self.nc = nc
        self.bytes = {"sync": 0, "gpsimd": 0}

    def dma(self, out, in_, nbytes, queue=None):
        if queue is None:
            queue = min(self.bytes, key=self.bytes.get)
        if queue in self.bytes:
            self.bytes[queue] += nbytes
        eng = getattr(self.nc, queue)
        eng.dma_start(out=out, in_=in_)


def _build():
    if "nc" in _CACHE:
        return _CACHE["nc"]
    nc = bacc.Bacc(
        "TRN2",
        target_bir_lowering=False,
        debug=False,
        enable_asserts=False,
        num_devices=NCORES,
    )
    # K byte-blob per couple: per partition [pair0 hi f16 | pair0 lo e3m4 |
    # pair1 hi | pair1 lo]; hi = MK chunks fp16, lo = ML chunks e3m4.
    kb_d = nc.dram_tensor("kb", [G, 128, 2 * KPB], F8E3, kind="ExternalInput").ap()
    v_d = nc.dram_tensor("vx", [G, 128, 2 * TC * D], F8E3, kind="ExternalInput").ap()
    w_d = nc.dram_tensor("wqkv", [3, HPC, 128, CCH, D], F16, kind="ExternalInput").ap()
    wo_d = nc.dram_tensor("wo", [HPC, D, C], F16, kind="ExternalInput").ap()
    it_d = nc.dram_tensor("inpt", [128, CCH, N], F16, kind="ExternalInput").ap()
    out_d = nc.dram_tensor("out", [128, CCH, N], F32, kind="ExternalOutput").ap()

    with tile.TileContext(nc) as tc:
        with (
            tc.tile_pool(name="const", bufs=1) as const,
            tc.tile_pool(name="kv", bufs=4) as kvpool,
            tc.tile_pool(name="small", bufs=2) as small,
            tc.tile_pool(name="ypool", bufs=2) as ypool,
            tc.tile_pool(name="opool", bufs=1) as opool,
            tc.tile_pool(name="ps", bufs=2, space="PSUM") as ps,
            tc.tile_pool(name="wops", bufs=1, space="PSUM") as wops,
        ):
            ones_col = const.tile([128, 1], F32)
            nc.vector.memset(ones_col[:], 1.0)
            ones_row = const.tile([1, 128], F32)
            nc.vector.memset(ones_row[:], 1.0)

            qb = _QueueBalancer(nc)
            WB = 128 * CCH * D * 2  # one w chunk [128, CCH, D] fp16

            # w_q + input lead on the KV queues so q projections start
            # ASAP; the rest of the weights ride the scalar (ACT) HWDGE
            # ring — issued before any exp instruction exists, so they
            # never starve a KV ring.
            # K rides sync alone; V + weights ride gpsimd (byte-balanced
            # ~20.5 MB each) so the two issue streams' pool-waits are
            # decorrelated — one parked engine can't dry the other ring.
            w_sb = const.tile([128, HPC, 3, CCH, D], F16)
            inpt_sb = const.tile([128, CCH, N], F16)
            wo_sb = const.tile([128, HPC, C], F16)
            for h in range(HPC):
                qb.dma(w_sb[:, h, 0], w_d[0, h], WB, queue="gpsimd")
            qb.dma(inpt_sb[:], it_d, 128 * CCH * N * 2, queue="gpsimd")

            def _late_weights(g):
                if g == 1:
                    for h in range(HPC):
                        qb.dma(w_sb[:, h, 1], w_d[1, h], WB, queue="gpsimd")
                elif g == 2:
                    for h in range(HPC):
                        qb.dma(w_sb[:, h, 2], w_d[2, h], WB, queue="gpsimd")
                elif g == 3:
                    for h in range(HPC):
                        qb.dma(
                            wo_sb[:, h, :], wo_d[h], 128 * C * 2,
                            queue="gpsimd",
                        )

            # q projections only; k_new/v_new projections run in the head
            # epilogues so the per-pair stream never waits on w_k/w_v.
            def _proj(h, w, dt, tag):
                # allocated [128, 2N] so every "pm"-tag tile has one size
                # (bc_ps/sn_ps reuse the tag at [128, 2N])
                pp = ps.tile([128, 2 * N], F32, tag="pm", bufs=1, name="pp")
                for cc in range(CCH):
                    nc.tensor.matmul(
                        pp[:, :N],
                        lhsT=w_sb[:, h, w, cc, :],
                        rhs=inpt_sb[:, cc, :],
                        start=(cc == 0),
                        stop=(cc == CCH - 1),
                    )
                sb = small.tile([128, N], dt, tag=tag, name=tag)
                nc.vector.tensor_copy(out=sb[:], in_=pp[:, :N])
                return sb

            qTs = [_proj(h, 0, F16, f"q{h}") for h in range(HPC)]

            wo_acc2 = wops.tile([128, HPC, CCH, N], F32, tag="woa", name="woa")
            wo_acc = [wo_acc2[:, h] for h in range(HPC)]
            kv_tiles: dict = {}
            heads = [
                {
                    "qT": qTs[h],
                    "den": ps.tile(
                        [1, N], F32, tag=f"den{h}", bufs=1, name=f"den{h}"
                    ),
                    "y": ypool.tile([128, N], F32, tag=f"y{h}", name=f"y{h}"),
                }
                for h in range(HPC)
            ]

            def _phase_a(p):
                """DMA + QK + exp issue for pair p; returns state for B."""
                h, n = divmod(p, N)
                g, gi = p // 2, p % 2
                qT_sb = heads[h]["qT"]

                if g < G - 1:
                    if gi == 0:
                        kraw = kvpool.tile(
                            [128, 2 * KPB], F8E3, tag="kraw", bufs=4,
                            name="kraw",
                        )
                        qb.dma(kraw[:], kb_d[g], 128 * 2 * KPB, queue="sync")
                        v2 = kvpool.tile(
                            [128, 2, TC, D], F8E3, tag="v8", bufs=3, name="v8"
                        )
                        qb.dma(
                            v2[:], v_d[g], 128 * 2 * TC * D, queue="gpsimd"
                        )
                        _late_weights(g)
                        kv_tiles[g] = (kraw, v2)
                    kraw, v_sb = kv_tiles[g]
                    kbase, vsl = gi * KPB, gi
                else:
                    # last couple: per-pair tiles/DMAs so the final pair's
                    # compute overlaps the final pair's bytes
                    kraw = kvpool.tile(
                        [128, KPB], F8E3, tag="krawL", bufs=2, name="krawL"
                    )
                    qb.dma(
                        kraw[:], kb_d[g, :, gi * KPB : (gi + 1) * KPB],
                        128 * KPB, queue="sync",
                    )
                    v_sb = kvpool.tile(
                        [128, 1, TC, D], F8E3, tag="v8L", bufs=2, name="v8L"
                    )
                    qb.dma(
                        v_sb[:],
                        v_d[g, :, gi * TC * D : (gi + 1) * TC * D],
                        128 * TC * D, queue="gpsimd",
                    )
                    kbase, vsl = 0, 0

                sc = ps.tile([128, TC], F32, tag="sc")
                for c in range(TC):
                    if c < MK:
                        off = kbase + c * 256
                        lhs = kraw[:, off : off + 256].bitcast(F16)
                    else:
                        off = kbase + MK * 256 + (c - MK) * 128
                        lhs = kraw[:, off : off + 128]
                    nc.tensor.matmul(
                        sc[:, c : c + 1],
                        lhsT=lhs,
                        rhs=qT_sb[:, n : n + 1],
                        start=True,
                        stop=True,
                    )

                attn = small.tile([128, TC], F16, tag="attn")
                acc = small.tile([128, 1], F32, tag="acc")
                nc.scalar.activation(
                    out=attn[:],
                    in_=sc[:],
                    func=mybir.ActivationFunctionType.Exp,
                    scale=SCALE,
                    accum_out=acc[:],
                )
                return (h, n, attn, acc, v_sb, vsl)

            def _phase_b(state):
                """den + PV + y copy for a pair whose exp already issued."""
                h, n, attn, acc, v_sb, vsl = state
                nc.tensor.matmul(
                    heads[h]["den"][0:1, n : n + 1],
                    lhsT=ones_col[:],
                    rhs=acc[:],
                    start=True,
                    stop=True,
                )
                y_ps = ps.tile([128, 1], F32, tag="yps")
                for c in range(TC):
                    nc.tensor.matmul(
                        y_ps[:],
                        lhsT=v_sb[:, vsl, c, :],
                        rhs=attn[:, c : c + 1],
                        start=(c == 0),
                        stop=(c == TC - 1),
                    )
                nc.vector.tensor_copy(
                    out=heads[h]["y"][:, n : n + 1], in_=y_ps[:]
                )
                if n == N - 1:
                    _epilogue(h)

            projs_kv: dict = {}

            def _epilogue(h):
                qT_sb = heads[h]["qT"]
                den_ps = heads[h]["den"]
                y_sb = heads[h]["y"]
                knT_sb, vnT_sb = projs_kv[h]

                # s_new[n] = k_new[:,n] . q[:,n] via DVE product + ones sum
                prod = ypool.tile([128, N], F32, tag="prod")
                nc.vector.tensor_mul(out=prod[:], in0=knT_sb[:], in1=qT_sb[:])
                sn_ps = ps.tile([128, 2 * N], F32, tag="pm", bufs=1, name="snps")
                nc.tensor.matmul(
                    sn_ps[0:1, :N], lhsT=ones_col[:], rhs=prod[:],
                    start=True, stop=True,
                )
                en_sb = small.tile([1, N], F32, tag="en")
                nc.scalar.activation(
                    out=en_sb[:],
                    in_=sn_ps[0:1, :N],
                    func=mybir.ActivationFunctionType.Exp,
                    scale=SCALE,
                )
                # den_tot = den + exp_new; inv = 1/den_tot; eninv = en*inv
                den_sb = small.tile([1, N], F32, tag="densb")
                nc.vector.tensor_copy(out=den_sb[:], in_=den_ps[:])
                nc.vector.tensor_add(out=den_sb[:], in0=den_sb[:], in1=en_sb[:])
                inv_sb = small.tile([1, N], F32, tag="inv")
                nc.vector.reciprocal(inv_sb[:], den_sb[:])
                eninv = small.tile([1, N], F32, tag="eninv")
                nc.vector.tensor_mul(out=eninv[:], in0=en_sb[:], in1=inv_sb[:])
                # broadcast rows across partitions; y_fin = y*inv + vn*eninv
                bc_ps = ps.tile([128, 2 * N], F32, tag="pm", bufs=1, name="bcps")
                nc.tensor.matmul(
                    bc_ps[:, :N], lhsT=ones_row[:], rhs=inv_sb[:],
                    start=True, stop=True,
                )
                nc.tensor.matmul(
                    bc_ps[:, N:], lhsT=ones_row[:], rhs=eninv[:],
                    start=True, stop=True,
                )
                tmp_y = ypool.tile([128, N], F32, tag="tmpy")
                nc.vector.tensor_mul(out=tmp_y[:], in0=vnT_sb[:], in1=bc_ps[:, N:])
                nc.vector.tensor_mul(out=y_sb[:], in0=y_sb[:], in1=bc_ps[:, :N])
                nc.vector.tensor_add(out=y_sb[:], in0=y_sb[:], in1=tmp_y[:])
                y16 = ypool.tile([128, N], F16, tag="y16")
                nc.vector.tensor_copy(out=y16[:], in_=y_sb[:])

                # out^T partial: wo_acc[h][c, n] = sum_d w_o[h, d, c] y16[d, n]
                for cc in range(CCH):
                    nc.tensor.matmul(
                        wo_acc[h][:, cc, :],
                        lhsT=wo_sb[:, h, cc * 128 : (cc + 1) * 128],
                        rhs=y16[:],
                        start=True,
                        stop=True,
                    )

            for p in range(P):
                _phase_b(_phase_a(p))
                # k_new/v_new projections mid-stream (weights have landed,
                # PE has slack) so the h1 epilogue isn't gated on them
                if p == 8:
                    for h in range(HPC):
                        projs_kv[h] = [_proj(h, 1, F16, f"kn{h}")]
                elif p == 10:
                    for h in range(HPC):
                        projs_kv[h].append(_proj(h, 2, F32, f"vn{h}"))

            outT = opool.tile([128, CCH, N], F32)
            nc.vector.tensor_copy(out=outT[:], in_=wo_acc[0])
            nc.vector.tensor_add(out=outT[:], in0=outT[:], in1=wo_acc[1])
            qb.dma(out_d, outT[:], 128 * CCH * N * 4, queue="scalar")

    nc.compile()
    _CACHE["nc"] = nc
    return nc


def shard_inputs(input, k_cache, v_cache, w_q, w_k, w_v, w_o):
    """Host-side layout/dtype prep: per-core input dicts."""
    input = np.asarray(input, dtype=np.float32)
    k_cache = np.asarray(k_cache, dtype=np.float32)
    v_cache = np.asarray(v_cache, dtype=np.float32)
    w_q = np.asarray(w_q, dtype=np.float32)
    w_k = np.asarray(w_k, dtype=np.float32)
    w_v = np.asarray(w_v, dtype=np.float32)
    w_o = np.asarray(w_o, dtype=np.float32)

    inpT = input.reshape(N, C).T  # [C, N]
    it_np = np.ascontiguousarray(
        inpT.reshape(CCH, 128, N).transpose(1, 0, 2)
    ).astype(np.float16)
    wo4 = w_o.reshape(H, D, C)
    wqkv = np.stack([w_q, w_k, w_v])  # [3, H, D, C]

    in_maps = []
    for core in range(NCORES):
        h0 = core * HPC
        # K^T per pair p = h*N + n: [P, D, T0]; blob per couple/partition:
        # [pair0 hi f16 | pair0 lo e3m4 | pair1 hi | pair1 lo]
        kT = k_cache[:, h0 : h0 + HPC].transpose(1, 0, 3, 2).reshape(P, D, T0)
        kh = np.ascontiguousarray(kT[:, :, : MK * 128]).astype(np.float16)
        kl = np.ascontiguousarray(kT[:, :, MK * 128 :]).astype(
            ml_dtypes.float8_e3m4
        )
        kb = np.empty((P, D, KPB), dtype=np.uint8)
        kb[:, :, : MK * 256] = kh.view(np.uint8)
        kb[:, :, MK * 256 :] = kl.view(np.uint8)
        kb = (
            kb.reshape(G, 2, D, KPB)
            .transpose(0, 2, 1, 3)
            .reshape(G, 128, 2 * KPB)
        )
        kb = np.ascontiguousarray(kb).view(ml_dtypes.float8_e3m4)
        # V packed per couple: [G, 128, 2, TC, D] e3m4 where
        # [g, pp, i, c, j] = V_{p=2g+i}[c*128+pp, j]
        v_np = (
            v_cache[:, h0 : h0 + HPC]
            .transpose(1, 0, 2, 3)            # [HPC, N, T0, DV]
            .reshape(P, TC, 128, D)
            .transpose(0, 2, 1, 3)            # [P, 128, TC, D]
            .reshape(G, 2, 128, TC, D)
            .transpose(0, 2, 1, 3, 4)         # [G, 128, 2, TC, D]
            .reshape(G, 128, 2 * TC * D)
        )
        v_np = np.ascontiguousarray(v_np).astype(ml_dtypes.float8_e3m4)
        # wT chunks: [3, HPC, 128, CCH, D]; wT[h] = w[h].T of shape [C, D]
        w_np = np.ascontiguousarray(
            wqkv[:, h0 : h0 + HPC]
            .transpose(0, 1, 3, 2)  # [3, HPC, C, D]
            .reshape(3, HPC, CCH, 128, D)
            .transpose(0, 1, 3, 2, 4)
        ).astype(np.float16)  # [3, HPC, 128, CCH, D]
        wo_np = np.ascontiguousarray(wo4[h0 : h0 + HPC]).astype(np.float16)
        in_maps.append(
            {
                "kb": kb,
                "vx": v_np,
                "wqkv": w_np,
                "wo": wo_np,
                "inpt": it_np,
            }
        )
    return in_maps


def _run(inputs: dict, trace: bool = False):
    nc = _build()
    in_maps = shard_inputs(**inputs)
    res = run_bass_kernel_spmd(
        nc, in_maps, core_ids=list(range(NCORES)), trace=trace
    )
    # out DRAM is out^T chunks: [128, CCH, N] with c = cc*128 + p
    acc = np.zeros((N, C), dtype=np.float64)
    for r in res.results:
        o = r["out"].reshape(128, CCH, N)
        acc += o.transpose(2, 1, 0).reshape(N, C)
    out = acc.astype(np.float32).reshape(N, 1, C)
    return out, res


def kernel(**inputs) -> np.ndarray:
    out, _ = _run(inputs, trace=False)
    return out


# revision 41
# speedup vs baseline: 1.0260x; 1.0197x over previous
"""Trainium2 Bass kernel for single-token-decode MHA with KV cache.

Problem: N=16, H=16, T0=4096, DQK=DV=128, DIM_IN=2048, fp32 inputs.
Sharding: head (tensor) parallelism across 8 cores — 2 heads per core, all
batches. Each core computes its 2 heads' attention plus the partial w_o
projection; the host sums the 8 partials (the "all-reduce after w_o").

The kernel is HBM-bound; optimization = fewer HBM bytes + tight streaming:
  - Weights/input fp16; K cache split by sequence chunk: first MK=12 of 32
    chunks fp16, the rest fp8 e3m4 (Trainium FP8_EXP3, 4 mantissa bits);
    V cache entirely e3m4. Host-side numpy sim of the exact pipeline puts
    rel err at 1.59e-2 (gate 2e-2; HW tracks the sim to ~1e-6).
  - Per-core HBM: 22.5 MB K + 16 MB V + 4.3 MB weights+input = 42.8 MB
    -> ~120 us floor at the ~358 GB/s per-NC HBM limit.
  - Transfer sizes kept at the DMA knee: K hi+lo bytes for a 2-pair couple
    ride ONE 1.41 MB byte-blob DMA (fp16 chunk views via AP.bitcast), V
    for the couple is one 1 MB DMA.
  - Queue plan: sync (HWDGE) + gpsimd (SWDGE) stream KV, greedily
    byte-balanced; the scalar engine's queue carries only the 4.3 MB of
    weights at the head, so its exp-activation semaphore waits can never
    starve a KV ring.
  - The new-token (k_new/v_new) path is folded into the per-head epilogue
    (DVE product + ones-column PE sum -> exp), so the per-pair stream
    depends only on w_q/input and the K/V tiles.
  - w_o stage runs transposed: out^T[c,n] accumulates per head in PSUM via
    lhsT=w_o chunks, rhs=y*(1/den); one [128, 256] copy + single output
    DMA at the end.
Compute per pair: 32 chunked QK^T matmuls (K chunk as lhsT, fp16 or e3m4
per chunk), exp+accum on ACT, 32 chained PV matmuls (V e3m4 lhsT x fp16
attn rhs), all accumulation in fp32 PSUM.
"""

import math

import numpy as np
import ml_dtypes

import concourse.bacc as bacc
import concourse.mybir as mybir
import concourse.tile as tile
from concourse.bass_utils import run_bass_kernel_spmd

N, H, T0, D, C = 16, 16, 4096, 128, 2048
NCORES = 8
HPC = H // NCORES          # heads per core = 2
TC = T0 // 128             # 32 sequence chunks of 128
MK = 6                     # K chunks kept in fp16 (rest e3m4)
ML = TC - MK
CCH = C // 128             # 16 contraction chunks of 128
P = HPC * N                # 32 (head, batch) pairs per core
G = P // 2                 # 16 pair-couples (K/V packed 2 pairs per DMA)
KPB = MK * 128 * 2 + ML * 128  # K bytes per pair per partition (5632)
SCALE = 1.0 / math.sqrt(D)

F32 = mybir.dt.float32
F16 = mybir.dt.float16
F8E3 = mybir.dt.float8e3

_CACHE: dict = {}


class _QueueBalancer:
    """Greedy compile-time byte balancing between the two KV DMA queues."""

    def __init__(self, nc):
        self.nc = nc
        self.bytes = {"sync": 0, "gpsimd": 0}

    def dma(self, out, in_, nbytes, queue=None):
        if queue is None:
            queue = min(self.bytes, key=self.bytes.get)
        if queue in self.bytes:
            self.bytes[queue] += nbytes
        eng = getattr(self.nc, queue)
        eng.dma_start(out=out, in_=in_)


def _build():
    if "nc" in _CACHE:
        return _CACHE["nc"]
    nc = bacc.Bacc(
        "TRN2",
        target_bir_lowering=False,
        debug=False,
        enable_asserts=False,
        num_devices=NCORES,
    )
    # K byte-blob per couple: per partition [pair0 hi f16 | pair0 lo e3m4 |
    # pair1 hi | pair1 lo]; hi = MK chunks fp16, lo = ML chunks e3m4.
    kb_d = nc.dram_tensor("kb", [G, 128, 2 * KPB], F8E3, kind="ExternalInput").ap()
    v_d = nc.dram_tensor("vx", [G, 128, 2 * TC * D], F8E3, kind="ExternalInput").ap()
    w_d = nc.dram_tensor("wqkv", [3, HPC, 128, CCH, D], F16, kind="ExternalInput").ap()
    wo_d = nc.dram_tensor("wo", [HPC, D, C], F16, kind="ExternalInput").ap()
    it_d = nc.dram_tensor("inpt", [128, CCH, N], F16, kind="ExternalInput").ap()
    out_d = nc.dram_tensor("out", [128, CCH, N], F32, kind="ExternalOutput").ap()

    with tile.TileContext(nc) as tc:
        with (
            tc.tile_pool(name="const", bufs=1) as const,
            tc.tile_pool(name="kv", bufs=4) as kvpool,
            tc.tile_pool(name="small", bufs=2) as small,
            tc.tile_pool(name="ypool", bufs=2) as ypool,
            tc.tile_pool(name="opool", bufs=1) as opool,
            tc.tile_pool(name="ps", bufs=2, space="PSUM") as ps,
            tc.tile_pool(name="wops", bufs=1, space="PSUM") as wops,
        ):
            ones_col = const.tile([128, 1], F32)
            nc.vector.memset(ones_col[:], 1.0)
            ones_row = const.tile([1, 128], F32)
            nc.vector.memset(ones_row[:], 1.0)

            qb = _QueueBalancer(nc)
            WB = 128 * CCH * D * 2  # one w chunk [128, CCH, D] fp16

            # w_q + input lead on the KV queues so q projections start
            # ASAP; the rest of the weights ride the scalar (ACT) HWDGE
            # ring — issued before any exp instruction exists, so they
            # never starve a KV ring.
            # K rides sync alone; V + weights ride gpsimd (byte-balanced
            # ~20.5 MB each) so the two issue streams' pool-waits are
            # decorrelated — one parked engine can't dry the other ring.
            w_sb = const.tile([128, HPC, 3, CCH, D], F16)
            inpt_sb = const.tile([128, CCH, N], F16)
            wo_sb = const.tile([128, HPC, C], F16)
            for h in range(HPC):
                qb.dma(w_sb[:, h, 0], w_d[0, h], WB, queue="sync")
            qb.dma(inpt_sb[:], it_d, 128 * CCH * N * 2, queue="sync")

            def _late_weights(g):
                if g == 1:
                    for h in range(HPC):
                        qb.dma(w_sb[:, h, 1], w_d[1, h], WB, queue="sync")
                elif g == 2:
                    for h in range(HPC):
                        qb.dma(w_sb[:, h, 2], w_d[2, h], WB, queue="sync")
                elif g == 3:
                    for h in range(HPC):
                        qb.dma(
                            wo_sb[:, h, :], wo_d[h], 128 * C * 2,
                            queue="sync",
                        )

            # q projections only; k_new/v_new projections run in the head
            # epilogues so the per-pair stream never waits on w_k/w_v.
            def _proj(h, w, dt, tag):
                # allocated [128, 2N] so every "pm"-tag tile has one size
                # (bc_ps/sn_ps reuse the tag at [128, 2N])
                pp = ps.tile([128, 2 * N], F32, tag="pm", bufs=1, name="pp")
                for cc in range(CCH):
                    nc.tensor.matmul(
                        pp[:, :N],
                        lhsT=w_sb[:, h, w, cc, :],
                        rhs=inpt_sb[:, cc, :],
                        start=(cc == 0),
                        stop=(cc == CCH - 1),
                    )
                sb = small.tile([128, N], dt, tag=tag, name=tag)
                nc.vector.tensor_copy(out=sb[:], in_=pp[:, :N])
                return sb

            qTs = [_proj(h, 0, F16, f"q{h}") for h in range(HPC)]

            wo_acc2 = wops.tile([128, HPC, CCH, N], F32, tag="woa", name="woa")
            wo_acc = [wo_acc2[:, h] for h in range(HPC)]
            kv_tiles: dict = {}
            heads = [
                {
                    "qT": qTs[h],
                    "den": ps.tile(
                        [1, N], F32, tag=f"den{h}", bufs=1, name=f"den{h}"
                    ),
                    "y": ypool.tile([128, N], F32, tag=f"y{h}", name=f"y{h}"),
                }
                for h in range(HPC)
            ]

            def _phase_a(p):
                """DMA + QK + exp issue for pair p; returns state for B."""
                h, n = divmod(p, N)
                g, gi = p // 2, p % 2
                qT_sb = heads[h]["qT"]

                if g < G - 1:
                    if gi == 0:
                        kraw = kvpool.tile(
                            [128, 2 * KPB], F8E3, tag="kraw", bufs=4,
                            name="kraw",
                        )
                        qb.dma(kraw[:], kb_d[g], 128 * 2 * KPB, queue="gpsimd")
                        v2 = kvpool.tile(
                            [128, 2, TC, D], F8E3, tag="v8", bufs=3, name="v8"
                        )
                        qb.dma(
                            v2[:], v_d[g], 128 * 2 * TC * D, queue="sync"
                        )
                        _late_weights(g)
                        kv_tiles[g] = (kraw, v2)
                    kraw, v_sb = kv_tiles[g]
                    kbase, vsl = gi * KPB, gi
                else:
                    # last couple: per-pair tiles/DMAs so the final pair's
                    # compute overlaps the final pair's bytes
                    kraw = kvpool.tile(
                        [128, KPB], F8E3, tag="krawL", bufs=2, name="krawL"
                    )
                    qb.dma(
                        kraw[:], kb_d[g, :, gi * KPB : (gi + 1) * KPB],
                        128 * KPB, queue="gpsimd",
                    )
                    v_sb = kvpool.tile(
                        [128, 1, TC, D], F8E3, tag="v8L", bufs=2, name="v8L"
                    )
                    qb.dma(
                        v_sb[:],
                        v_d[g, :, gi * TC * D : (gi + 1) * TC * D],
                        128 * TC * D, queue="sync",
                    )
                    kbase, vsl = 0, 0

                sc = ps.tile([128, TC], F32, tag="sc")
                for c in range(TC):
                    if c < MK:
                        off = kbase + c * 256
                        lhs = kraw[:, off : off + 256].bitcast(F16)
                    else:
                        off = kbase + MK * 256 + (c - MK) * 128
                        lhs = kraw[:, off : off + 128]
                    nc.tensor.matmul(
                        sc[:, c : c + 1],
                        lhsT=lhs,
                        rhs=qT_sb[:, n : n + 1],
                        start=True,
                        stop=True,
                    )

                attn = small.tile([128, TC], F16, tag="attn")
                acc = small.tile([128, 1], F32, tag="acc")
                nc.scalar.activation(
                    out=attn[:],
                    in_=sc[:],
                    func=mybir.ActivationFunctionType.Exp,
                    scale=SCALE,
                    accum_out=acc[:],
                )
                return (h, n, attn, acc, v_sb, vsl)

            def _phase_b(state):
                """den + PV + y copy for a pair whose exp already issued."""
                h, n, attn, acc, v_sb, vsl = state
                nc.tensor.matmul(
                    heads[h]["den"][0:1, n : n + 1],
                    lhsT=ones_col[:],
                    rhs=acc[:],
                    start=True,
                    stop=True,
                )
                y_ps = ps.tile([128, 1], F32, tag="yps")
                for c in range(TC):
                    nc.tensor.matmul(
                        y_ps[:],
                        lhsT=v_sb[:, vsl, c, :],
                        rhs=attn[:, c : c + 1],
                        start=(c == 0),
                        stop=(c == TC - 1),
                    )
                nc.vector.tensor_copy(
                    out=heads[h]["y"][:, n : n + 1], in_=y_ps[:]
                )
                if n == N - 1:
                    _epilogue(h)

            projs_kv: dict = {}

            def _epilogue(h):
                qT_sb = heads[h]["qT"]
                den_ps = heads[h]["den"]
                y_sb = heads[h]["y"]
                knT_sb, vnT_sb = projs_kv[h]

                # s_new[n] = k_new[:,n] . q[:,n] via DVE product + ones sum
                prod = ypool.tile([128, N], F32, tag="prod")
                nc.vector.tensor_mul(out=prod[:], in0=knT_sb[:], in1=qT_sb[:])
                sn_ps = ps.tile([128, 2 * N], F32, tag="pm", bufs=1, name="snps")
                nc.tensor.matmul(
                    sn_ps[0:1, :N], lhsT=ones_col[:], rhs=prod[:],
                    start=True, stop=True,
                )
                en_sb = small.tile([1, N], F32, tag="en")
                nc.scalar.activation(
                    out=en_sb[:],
                    in_=sn_ps[0:1, :N],
                    func=mybir.ActivationFunctionType.Exp,
                    scale=SCALE,
                )
                # den_tot = den + exp_new; inv = 1/den_tot; eninv = en*inv
                den_sb = small.tile([1, N], F32, tag="densb")
                nc.vector.tensor_copy(out=den_sb[:], in_=den_ps[:])
                nc.vector.tensor_add(out=den_sb[:], in0=den_sb[:], in1=en_sb[:])
                inv_sb = small.tile([1, N], F32, tag="inv")
                nc.vector.reciprocal(inv_sb[:], den_sb[:])
                eninv = small.tile([1, N], F32, tag="eninv")
                nc.vector.tensor_mul(out=eninv[:], in0=en_sb[:], in1=inv_sb[:])
                # broadcast rows across partitions; y_fin = y*inv + vn*eninv
                bc_ps = ps.tile([128, 2 * N], F32, tag="pm", bufs=1, name="bcps")
                nc.tensor.matmul(
                    bc_ps[:, :N], lhsT=ones_row[:], rhs=inv_sb[:],
                    start=True, stop=True,
                )
                nc.tensor.matmul(
                    bc_ps[:, N:], lhsT=ones_row[:], rhs=eninv[:],
                    start=True, stop=True,
                )
                tmp_y = ypool.tile([128, N], F32, tag="tmpy")
                nc.vector.tensor_mul(out=tmp_y[:], in0=vnT_sb[:], in1=bc_ps[:, N:])
                nc.vector.tensor_mul(out=y_sb[:], in0=y_sb[:], in1=bc_ps[:, :N])
                nc.vector.tensor_add(out=y_sb[:], in0=y_sb[:], in1=tmp_y[:])
                y16 = ypool.tile([128, N], F16, tag="y16")
                nc.vector.tensor_copy(out=y16[:], in_=y_sb[:])

                # out^T partial: wo_acc[h][c, n] = sum_d w_o[h, d, c] y16[d, n]
                for cc in range(CCH):
                    nc.tensor.matmul(
                        wo_acc[h][:, cc, :],
                        lhsT=wo_sb[:, h, cc * 128 : (cc + 1) * 128],
                        rhs=y16[:],
                        start=True,
                        stop=True,
                    )

            for p in range(P):
                _phase_b(_phase_a(p))
                # k_new/v_new projections mid-stream (weights have landed,
                # PE has slack) so the h1 epilogue isn't gated on them
                if p == 8:
                    for h in range(HPC):
                        projs_kv[h] = [_proj(h, 1, F16, f"kn{h}")]
                elif p == 10:
                    for h in range(HPC):
                        projs_kv[h].append(_proj(h, 2, F32, f"vn{h}"))

            outT = opool.tile([128, CCH, N], F32)
            nc.vector.tensor_copy(out=outT[:], in_=wo_acc[0])
            nc.vector.tensor_add(out=outT[:], in0=outT[:], in1=wo_acc[1])
            qb.dma(out_d, outT[:], 128 * CCH * N * 4, queue="scalar")

    nc.compile()
    _CACHE["nc"] = nc
    return nc


def shard_inputs(input, k_cache, v_cache, w_q, w_k, w_v, w_o):
    """Host-side layout/dtype prep: per-core input dicts."""
    input = np.asarray(input, dtype=np.float32)
    k_cache = np.asarray(k_cache, dtype=np.float32)
    v_cache = np.asarray(v_cache, dtype=np.float32)
    w_q = np.asarray(w_q, dtype=np.float32)
    w_k = np.asarray(w_k, dtype=np.float32)
    w_v = np.asarray(w_v, dtype=np.float32)
    w_o = np.asarray(w_o, dtype=np.float32)

    inpT = input.reshape(N, C).T  # [C, N]
    it_np = np.ascontiguousarray(
        inpT.reshape(CCH, 128, N).transpose(1, 0, 2)
    ).astype(np.float16)
    wo4 = w_o.reshape(H, D, C)
    wqkv = np.stack([w_q, w_k, w_v])  # [3, H, D, C]

    in_maps = []
    for core in range(NCORES):
        h0 = core * HPC
        # K^T per pair p = h*N + n: [P, D, T0]; blob per couple/partition:
        # [pair0 hi f16 | pair0 lo e3m4 | pair1 hi | pair1 lo]
        kT = k_cache[:, h0 : h0 + HPC].transpose(1, 0, 3, 2).reshape(P, D, T0)
        kh = np.ascontiguousarray(kT[:, :, : MK * 128]).astype(np.float16)
        kl = np.ascontiguousarray(kT[:, :, MK * 128 :]).astype(
            ml_dtypes.float8_e3m4
        )
        kb = np.empty((P, D, KPB), dtype=np.uint8)
        kb[:, :, : MK * 256] = kh.view(np.uint8)
        kb[:, :, MK * 256 :] = kl.view(np.uint8)
        kb = (
            kb.reshape(G, 2, D, KPB)
            .transpose(0, 2, 1, 3)
            .reshape(G, 128, 2 * KPB)
        )
        kb = np.ascontiguousarray(kb).view(ml_dtypes.float8_e3m4)
        # V packed per couple: [G, 128, 2, TC, D] e3m4 where
        # [g, pp, i, c, j] = V_{p=2g+i}[c*128+pp, j]
        v_np = (
            v_cache[:, h0 : h0 + HPC]
            .transpose(1, 0, 2, 3)            # [HPC, N, T0, DV]
            .reshape(P, TC, 128, D)
            .transpose(0, 2, 1, 3)            # [P, 128, TC, D]
            .reshape(G, 2, 128, TC, D)
            .transpose(0, 2, 1, 3, 4)         # [G, 128, 2, TC, D]
            .reshape(G, 128, 2 * TC * D)
        )
        v_np = np.ascontiguousarray(v_np).astype(ml_dtypes.float8_e3m4)
        # wT chunks: [3, HPC, 128, CCH, D]; wT[h] = w[h].T of shape [C, D]
        w_np = np.ascontiguousarray(
            wqkv[:, h0 : h0 + HPC]
            .transpose(0, 1, 3, 2)  # [3, HPC, C, D]
            .reshape(3, HPC, CCH, 128, D)
            .transpose(0, 1, 3, 2, 4)
        ).astype(np.float16)  # [3, HPC, 128, CCH, D]
        wo_np = np.ascontiguousarray(wo4[h0 : h0 + HPC]).astype(np.float16)
        in_maps.append(
            {
                "kb": kb,
                "vx": v_np,
                "wqkv": w_np,
                "wo": wo_np,
                "inpt": it_np,
            }
        )
    return in_maps


def _run(inputs: dict, trace: bool = False):
    nc = _build()
    in_maps = shard_inputs(**inputs)
    res = run_bass_kernel_spmd(
        nc, in_maps, core_ids=list(range(NCORES)), trace=trace
    )
    # out DRAM is out^T chunks: [128, CCH, N] with c = cc*128 + p
    acc = np.zeros((N, C), dtype=np.float64)
    for r in res.results:
        o = r["out"].reshape(128, CCH, N)
        acc += o.transpose(2, 1, 0).reshape(N, C)
    out = acc.astype(np.float32).reshape(N, 1, C)
    return out, res


def kernel(**inputs) -> np.ndarray:
    out, _ = _run(inputs, trace=False)
    return out
